# revision 42
# baseline (speedup 1.0000x reference)
"""MoE top-1 routing dispatch kernel for Trainium2 (8 NeuronCores, Bass/Tile).

Strategy (expert-parallel dispatch, token-parallel routing):
  - Tokens are sharded contiguously: core c owns tokens [c*2048, (c+1)*2048).
  - Each core computes router logits for its shard on the PE (fp32, 4-way
    col-group concurrency via tile_position), argmax routes, and within-shard
    per-expert running positions via cumsum matmuls.
  - Per-core per-expert counts are AllGathered (tiny) to turn local positions
    into global positions (the reference's global token-order cumsum).
  - Each core scatters (token_id+1) into a global slot table
    [E*capacity] at slot = route*capacity + global_pos (overflow tokens are
    dropped via DMA bounds-check) in ONE indirect DMA. A ReduceScatter(add)
    hands core c exactly its expert's 2048-slot segment.
  - Core c then gathers the 2048 source rows from a replicated bf16 copy of x
    (with an appended zero row for empty slots) via indirect DMA and writes
    its [2048, 4096] bf16 output shard. Host upcasts to f32 and concatenates.
"""

import os
import sys

import numpy as np
import ml_dtypes

for _p in ("/opt/trn_rl_repo", os.path.expanduser("~/.axon_site/_ro/trn_rl_repo")):
    if _p not in sys.path and os.path.isdir(_p):
        sys.path.insert(0, _p)

import concourse.bass as bass
import concourse.bacc as bacc
import concourse.tile as tile
from concourse import mybir
from concourse.bass_utils import run_bass_kernel_spmd
from concourse.masks import make_identity

F32 = mybir.dt.float32
F32R = mybir.dt.float32r
BF16 = mybir.dt.bfloat16
U32 = mybir.dt.uint32
I32 = mybir.dt.int32
AX = mybir.AxisListType
OP = mybir.AluOpType

P = 128
E = 8          # experts
M = 8          # cores
D = 4096       # hidden
CAP = 2048     # capacity per expert
N = M * CAP    # total tokens (16384)
NSH = N // M   # tokens per core (2048)
KT = D // P    # K slices (32)
TT = NSH // P  # token tiles per core (16)
NQ = 4         # 512-col chunks of the token dim for router matmul
QW = NSH // NQ # 512

BF = np.dtype(ml_dtypes.bfloat16)


def build_program(single=False):
    nc = bacc.Bacc(
        "TRN2",
        target_bir_lowering=False,
        debug=False,
        num_devices=M,
    )

    x_full = nc.dram_tensor("x_full", [N + 1, D], BF16, kind="ExternalInput")
    # chunk-major transposed x: [token-chunk, k-group, partition, 4*QW] so a
    # 1 MB load covers 4 k-slices of one 512-token chunk with 8 KB rows.
    xTq = nc.dram_tensor("xTq", [NQ, KT // 4, P, 4 * QW], F32, kind="ExternalInput")
    WT = nc.dram_tensor("WT", [P, KT * E], F32, kind="ExternalInput")
    bvecT = nc.dram_tensor("bvecT", [E, 1], F32, kind="ExternalInput")
    coremask = nc.dram_tensor("coremask", [M, P], F32, kind="ExternalInput")
    tok_ids = nc.dram_tensor("tok_ids", [P, TT], F32, kind="ExternalInput")
    out = nc.dram_tensor("out", [CAP, D], BF16, kind="ExternalOutput")

    groups = [list(range(M))]

    with tile.TileContext(nc) as tc:
        with (
            tc.tile_pool(name="const", bufs=1) as const,
            tc.tile_pool(name="xpool", bufs=3) as xpool,
            tc.tile_pool(name="meta", bufs=1) as meta,
            tc.tile_pool(name="rows", bufs=6) as rows,
            tc.tile_pool(name="ps_lg", bufs=1, space="PSUM") as ps_lg,
            tc.tile_pool(name="ps_m", bufs=1, space="PSUM") as ps_m,
            tc.tile_pool(name="dram", bufs=1, space="DRAM") as dram,
        ):
            # ---------------- constants ----------------
            ones = const.tile([P, P], F32, tag="ones")
            nc.gpsimd.memset(ones[:], 1.0)
            ident = const.tile([P, P], F32, tag="ident")
            make_identity(nc, ident[:])
            # U[p, f] = 1 iff p <= f  (inclusive-cumsum operator)
            U = const.tile([P, P], F32, tag="U")
            nc.gpsimd.affine_select(
                U[:], ones[:], pattern=[[1, P]], base=0,
                channel_multiplier=-1, compare_op=OP.is_ge, fill=0.0,
            )
            # A16[p, f] = 1 iff p < f  (exclusive-prefix operator over tiles)
            A16 = const.tile([TT, TT], F32, tag="A16")
            nc.gpsimd.affine_select(
                A16[:], ones[:TT, :TT], pattern=[[1, TT]], base=0,
                channel_multiplier=-1, compare_op=OP.is_gt, fill=0.0,
            )
            # expert-id iota repeated per tile: [128, TT, E] = e
            iota_i = const.tile([P, TT, E], I32, tag="iota_i")
            nc.gpsimd.iota(iota_i[:], pattern=[[0, TT], [1, E]], base=0, channel_multiplier=0)
            iota_f = const.tile([P, TT, E], F32, tag="iota_f")
            nc.vector.tensor_copy(iota_f[:], iota_i[:])
            iota_m16 = const.tile([P, TT, E], F32, tag="iota_m16")
            nc.vector.tensor_scalar(iota_m16[:], iota_f[:], -16.0, None, OP.add)

            wt_sb = const.tile([P, KT, E], F32, tag="wt")
            nc.sync.dma_start(
                wt_sb[:], WT.ap().rearrange("d (k e) -> d k e", e=E)
            )
            b_sb = const.tile([E, 1], F32, tag="b")
            nc.sync.dma_start(b_sb[:], bvecT.ap())
            cm_sb = const.tile([M, P], F32, tag="cm")
            nc.sync.dma_start(cm_sb[:], coremask.ap())
            tok_sb = const.tile([P, TT], F32, tag="tok")
            nc.sync.dma_start(tok_sb[:], tok_ids.ap())

            # ---------------- early: warm the collective path ----------------
            warm_in = dram.tile([1, E], F32, tag="warm_in")
            warm_out = dram.tile([M, E], F32, tag="warm_out")
            zrow = const.tile([1, E], F32, tag="zrow")
            nc.gpsimd.memset(zrow[:], 0.0)
            nc.gpsimd.dma_start(warm_in[:], zrow[:])
            if not single:
                nc.gpsimd.collective_compute(
                    "AllGather", OP.bypass, replica_groups=groups,
                    ins=[warm_in[:].opt()], outs=[warm_out[:].opt()],
                )

            # ---------------- early: zero the 16 slot sub-tables ----------------
            # HW indirect DMA supports only [128, 1] offset APs (one offset per
            # partition), so the 2048-token scatter is 16 calls. Independent
            # destination tables keep Tile from serializing them; they are
            # summed on-chip before the ReduceScatter.
            NTAB = 16
            table_in = dram.tile([N, 1], F32, tag="table_in")
            table_seg = dram.tile([CAP, 1], F32, tag="table_seg")
            tabs = [
                dram.tile([N, 1], F32, tag=f"tab{i}", name=f"tab{i}")
                for i in range(NTAB)
            ]
            zsb = meta.tile([P, P], F32, tag="zsb")
            nc.gpsimd.memset(zsb[:], 0.0)
            # zeroing runs on the scalar queue so it doesn't delay the
            # router's xTq loads on the sync queue
            for i in range(NTAB):
                nc.scalar.dma_start(
                    tabs[i][:].rearrange("(p f) v -> p (f v)", p=P), zsb[:]
                )
            # warm the ReduceScatter path too (reads the zeroed table)
            nc.scalar.dma_start(
                table_in[:].rearrange("(p f) v -> p (f v)", p=P), zsb[:]
            )
            warm_seg = dram.tile([CAP, 1], F32, tag="warm_seg")
            if not single:
                nc.gpsimd.collective_compute(
                    "ReduceScatter", OP.add, replica_groups=groups,
                    ins=[table_in[:].opt()], outs=[warm_seg[:].opt()],
                )

            # ---------------- router matmul: logits, chunk-pipelined ---------
            # Token-chunk-outer loop: chunk q's transpose/argmax/one-hot run
            # on PE/DVE while chunk q+1's matmuls stream, so the per-core
            # counts (and with them the AllGather) are ready right after the
            # last matmul instead of ~15us later.
            lg_ps = [
                ps_lg.tile([E, QW], F32, tag=f"lg{q}", name=f"lg{q}")
                for q in range(NQ)
            ]
            mg = meta.tile([E, NSH], F32, tag="mg")
            lgT_ps = ps_m.tile([P, TT, E], F32, tag="ps_big")
            lgT = meta.tile([P, TT, E], F32, tag="lgT_sb")
            mx = meta.tile([P, TT], F32, tag="mx")
            eq = meta.tile([P, TT, E], F32, tag="eq")
            v = meta.tile([P, TT, E], F32, tag="v")
            rmin = meta.tile([P, TT], F32, tag="rmin")
            route = meta.tile([P, TT], F32, tag="route")
            onehot = meta.tile([P, TT, E], F32, tag="onehot")

            def chunk_post(q):
                """merge+transpose+argmax+one-hot for token chunk q."""
                sl = slice(q * QW, (q + 1) * QW)
                ti = slice(4 * q, 4 * q + 4)
                nc.vector.tensor_scalar(
                    mg[:, sl], lg_ps[q][:], b_sb[:, 0:1], None, OP.add
                )
                for i in range(4 * q, 4 * q + 4):
                    nc.tensor.transpose(
                        lgT_ps[:, i, :], mg[:, i * P:(i + 1) * P], ident[:E, :E]
                    )
                nc.vector.tensor_copy(lgT[:, ti, :], lgT_ps[:, ti, :])
                nc.vector.tensor_reduce(
                    mx[:, ti], lgT[:, ti, :], axis=AX.X, op=OP.max
                )
                nc.vector.tensor_tensor(
                    eq[:, ti, :], lgT[:, ti, :],
                    mx[:, ti].unsqueeze(-1).broadcast_to((P, 4, E)), OP.is_equal,
                )
                nc.vector.tensor_tensor(
                    v[:, ti, :], eq[:, ti, :], iota_m16[:, ti, :], OP.mult
                )
                nc.vector.tensor_reduce(
                    rmin[:, ti], v[:, ti, :], axis=AX.X, op=OP.min
                )
                nc.vector.tensor_scalar(
                    route[:, ti], rmin[:, ti], 16.0, None, OP.add
                )
                nc.vector.tensor_tensor(
                    onehot[:, ti, :], iota_f[:, ti, :],
                    route[:, ti].unsqueeze(-1).broadcast_to((P, 4, E)),
                    OP.is_equal,
                )

            for q in range(NQ):
                for kg in range(KT // 4):
                    xkg = xpool.tile([P, 4 * QW], F32, tag="xkg")
                    nc.sync.dma_start(xkg[:], xTq.ap()[q][kg])
                    for j in range(4):
                        k = 4 * kg + j
                        nc.tensor.matmul(
                            lg_ps[q][:],
                            wt_sb[:, k, :],
                            xkg[:, j * QW:(j + 1) * QW],
                            start=(k == 0),
                            stop=(k == KT - 1),
                        )
                    if kg == 2 and q > 0:
                        chunk_post(q - 1)
            chunk_post(NQ - 1)

            # ---------------- per-(tile, e) totals ----------------
            totals_ps = ps_m.tile([1, TT * E], F32, tag="ps_small")
            nc.tensor.matmul(totals_ps[:], ones[:, :1], onehot[:], start=True, stop=True)
            totals_row = meta.tile([1, TT * E], F32, tag="totals_row")
            nc.vector.tensor_copy(totals_row[:], totals_ps[:])
            totals16 = meta.tile([TT, E], F32, tag="totals16")
            nc.sync.dma_start(totals16[:], totals_row[:])
            my_tot_ps = ps_m.tile([1, E], F32, tag="ps_small")
            nc.tensor.matmul(my_tot_ps[:], ones[:TT, :1], totals16[:], start=True, stop=True)
            my_tot = meta.tile([1, E], F32, tag="my_tot_sb")
            nc.vector.tensor_copy(my_tot[:], my_tot_ps[:])

            # AG-independent: LOCAL inclusive positions (within-tile cumsum +
            # tile prefix), fully computed before the collective.
            off16_ps = ps_m.tile([TT, E], F32, tag="ps_off")
            nc.tensor.matmul(off16_ps[:], A16[:], totals16[:], start=True, stop=True)
            off16 = meta.tile([TT, E], F32, tag="off16_sb")
            nc.vector.tensor_copy(off16[:], off16_ps[:])
            off_row = meta.tile([1, TT * E], F32, tag="off_row")
            nc.sync.dma_start(off_row[:], off16[:])
            cs2_ps = ps_m.tile([P, TT * E], F32, tag="ps_big")
            nc.tensor.matmul(cs2_ps[:], U[:], onehot[:], start=True, stop=False)
            nc.tensor.matmul(cs2_ps[:], ones[:1, :], off_row[:], start=False, stop=True)
            cs2_sb = meta.tile([P, TT, E], F32, tag="cs2_sb")
            nc.vector.tensor_copy(
                cs2_sb[:], cs2_ps[:].rearrange("p (i e) -> p i e", e=E)
            )
            sel = meta.tile([P, TT, E], F32, tag="sel")
            nc.vector.tensor_tensor(sel[:], cs2_sb[:], onehot[:], OP.mult)
            gincl_loc = meta.tile([P, TT], F32, tag="gincl_loc")
            nc.vector.tensor_reduce(gincl_loc[:], sel[:], axis=AX.X, op=OP.add)

            # ---------------- collective 1: AllGather per-core totals --------
            cc_in = dram.tile([1, E], F32, tag="cc_in")
            cc_out = dram.tile([M, E], F32, tag="cc_out")
            nc.gpsimd.dma_start(cc_in[:], my_tot[:])
            if single:
                nc.gpsimd.dma_start(cc_out[0:1, :], cc_in[:])
            else:
                nc.gpsimd.collective_compute(
                    "AllGather", OP.bypass, replica_groups=groups,
                    ins=[cc_in[:].opt()], outs=[cc_out[:].opt()],
                )
            totals_all = meta.tile([M, E], F32, tag="totals_all")
            nc.sync.dma_start(totals_all[:], cc_out[:])

            # per-token core offset: replicate the prefix row across all 128
            # partitions via the coremask matmul, then a broadcast lookup.
            core_off_ps = ps_m.tile([P, E], F32, tag="ps_small")
            nc.tensor.matmul(core_off_ps[:], cm_sb[:], totals_all[:], start=True, stop=True)
            core_off_b = meta.tile([P, E], F32, tag="core_off_b")
            nc.vector.tensor_copy(core_off_b[:], core_off_ps[:])
            tokoff_t = meta.tile([P, TT, E], F32, tag="tokoff_t")
            nc.vector.tensor_tensor(
                tokoff_t[:], onehot[:],
                core_off_b[:].unsqueeze(1).broadcast_to((P, TT, E)), OP.mult,
            )
            tokoff = meta.tile([P, TT], F32, tag="tokoff")
            nc.vector.tensor_reduce(tokoff[:], tokoff_t[:], axis=AX.X, op=OP.add)

            # ---------------- per-token slot computation ----------------
            gincl = meta.tile([P, TT], F32, tag="gincl")
            nc.vector.tensor_tensor(gincl[:], gincl_loc[:], tokoff[:], OP.add)
            gpos = meta.tile([P, TT], F32, tag="gpos")
            nc.vector.tensor_scalar(gpos[:], gincl[:], -1.0, None, OP.add)
            drop = meta.tile([P, TT], F32, tag="drop")
            nc.vector.tensor_scalar(drop[:], gincl[:], float(CAP), None, OP.is_gt)
            dslot = meta.tile([P, TT], F32, tag="dslot")
            nc.vector.scalar_tensor_tensor(
                dslot[:], route[:], float(CAP), gpos[:], op0=OP.mult, op1=OP.add
            )
            dslot_m = meta.tile([P, TT], F32, tag="dslot_m")
            nc.vector.scalar_tensor_tensor(
                dslot_m[:], drop[:], 100000.0, dslot[:], op0=OP.mult, op1=OP.add
            )
            dslot_u = meta.tile([P, TT], U32, tag="dslot_u")
            nc.vector.tensor_copy(dslot_u[:], dslot_m[:])

            # ---------------- slot-table scatter + merge + ReduceScatter ---------
            # 16 scatter calls round-robin over 8 tables; same-table WAW waits
            # are fully hidden behind the other 7 calls' descriptor generation.
            for i in range(TT):
                nc.gpsimd.indirect_dma_start(
                    out=tabs[i % NTAB][:],
                    out_offset=bass.IndirectOffsetOnAxis(ap=dslot_u[:, i:i + 1], axis=0),
                    in_=tok_sb[:, i:i + 1],
                    in_offset=None,
                    bounds_check=N - 1,
                    oob_is_err=False,
                )
            # merge: read each sub-table as [128, 128], sum, write table_in
            tsb = [
                meta.tile([P, P], F32, tag=f"tsb{i}", name=f"tsb{i}")
                for i in range(NTAB)
            ]
            for i in range(NTAB):
                weng = nc.sync if i % 2 == 0 else nc.scalar
                weng.dma_start(
                    tsb[i][:], tabs[i][:].rearrange("(p f) v -> p (f v)", p=P)
                )
            # ping-pong accumulate on DVE (~0.3us each)
            accA = meta.tile([P, P], F32, tag="accA")
            accB = meta.tile([P, P], F32, tag="accB")
            nc.vector.tensor_tensor(accA[:], tsb[0][:], tsb[1][:], OP.add)
            cur, nxt = accA, accB
            for i in range(2, NTAB):
                nc.vector.tensor_tensor(nxt[:], cur[:], tsb[i][:], OP.add)
                cur, nxt = nxt, cur
            nc.sync.dma_start(
                table_in[:].rearrange("(p f) v -> p (f v)", p=P), cur[:]
            )
            if single:
                nc.gpsimd.dma_start(table_seg[:], table_in[:CAP, :])
            else:
                nc.gpsimd.collective_compute(
                    "ReduceScatter", OP.add, replica_groups=groups,
                    ins=[table_in[:].opt()], outs=[table_seg[:].opt()],
                )

            # ---------------- gather dispatch ----------------
            tval = meta.tile([P, TT], F32, tag="tval")
            nc.sync.dma_start(
                tval[:], table_seg[:].rearrange("(i p) v -> p (i v)", p=P)
            )
            empty = meta.tile([P, TT], F32, tag="empty")
            nc.vector.tensor_scalar(empty[:], tval[:], 0.0, None, OP.is_equal)
            vm1 = meta.tile([P, TT], F32, tag="vm1")
            nc.vector.tensor_scalar(vm1[:], tval[:], -1.0, None, OP.add)
            gidx_f = meta.tile([P, TT], F32, tag="gidx_f")
            nc.vector.scalar_tensor_tensor(
                gidx_f[:], empty[:], float(N + 1), vm1[:], op0=OP.mult, op1=OP.add
            )
            gidx_u = meta.tile([P, TT], U32, tag="gidx_u")
            nc.vector.tensor_copy(gidx_u[:], gidx_f[:])

            for i in range(TT):
                xr = rows.tile([P, D], BF16, tag="xr")
                nc.gpsimd.indirect_dma_start(
                    out=xr[:],
                    out_offset=None,
                    in_=x_full.ap(),
                    in_offset=bass.IndirectOffsetOnAxis(ap=gidx_u[:, i:i + 1], axis=0),
                )
                weng = nc.sync if i % 2 == 0 else nc.scalar
                weng.dma_start(out.ap()[i * P:(i + 1) * P, :], xr[:])

    nc.compile()
    return nc


_CACHE = {}


def _get_program():
    if "nc" not in _CACHE:
        _CACHE["nc"] = build_program()
    return _CACHE["nc"]


def make_in_maps(x, W, b):
    x = np.ascontiguousarray(np.asarray(x, dtype=np.float32)).reshape(N, D)
    W = np.asarray(W, dtype=np.float32)
    b = np.asarray(b, dtype=np.float32)
    x_full = np.concatenate(
        [x.astype(BF), np.zeros((1, D), BF)], axis=0
    )
    WT_np = np.ascontiguousarray(
        W.T.reshape(KT, P, E).transpose(1, 0, 2)
    ).reshape(P, KT * E)
    b_np = b.reshape(E, 1)
    in_maps = []
    for c in range(M):
        shard = x[c * NSH:(c + 1) * NSH]
        xT_np = np.ascontiguousarray(shard.T).reshape(KT, P, NSH)
        # [KT, P, NSH] -> [NQ, KT//4, P, 4, QW] chunk-major
        xTq_np = np.ascontiguousarray(
            xT_np.reshape(KT // 4, 4, P, NQ, QW).transpose(3, 0, 2, 1, 4)
        ).reshape(NQ, KT // 4, P, 4 * QW)
        cm = np.repeat(
            (np.arange(M) < c).astype(np.float32).reshape(M, 1), P, axis=1
        )
        tok = (
            c * NSH + 1
            + np.arange(TT, dtype=np.float32)[None, :] * P
            + np.arange(P, dtype=np.float32)[:, None]
        ).astype(np.float32)
        in_maps.append(
            {
                "x_full": x_full,
                "xTq": xTq_np,
                "WT": WT_np,
                "bvecT": b_np,
                "coremask": cm,
                "tok_ids": tok,
            }
        )
    return in_maps


def kernel(x, W, b):
    nc = _get_program()
    in_maps = make_in_maps(x, W, b)
    res = run_bass_kernel_spmd(nc, in_maps, core_ids=list(range(M)))
    out = np.concatenate(
        [np.asarray(res.results[c]["out"]).astype(np.float32) for c in range(M)],
        axis=0,
    )
    return out
